# revision 41
# baseline (speedup 1.0000x reference)
"""Trainium2 Bass kernel for nn_MultiHeadAttention (B=2, S=2048, E=1024, H=16, D=64).

Sharding: 8 cores = 2 batches x 4 head-groups (4 heads / core, d_local=256).
Each core computes, for its (batch b, head group g):
    q = Xq[b] @ Wq[:, hs]*0.125 + bq[hs]*0.125        (transposed layout QT [256, S])
    k = Xk[b] @ Wk[:, hs] + bk[hs]                    (transposed layout KT [256, S])
    v = Xv[b] @ Wv[:, hs] + bv[hs]                    (natural layout, 65-strided + ones col)
    per head: scores^T = K_h @ Q_h^T  -> exp (ACT) -> Z|denom = expW^T.T @ [V_h|1]
    Z normalized per-partition, PE-transposed to ZT [256, S]
    partial_out = Z @ Wo[hs, :]                       ([S, E] fp32, host sums over g)
Host: transposes/casts inputs to bf16, sums the 4 partials per batch, adds bo.

Self-contained: hardcodes all shapes; requires only concourse (+ml_dtypes/numpy).
"""

import sys
import types

import numpy as np
import ml_dtypes

import concourse.bass as bass  # noqa: F401  (bass types used via tile/bacc)
import concourse.mybir as mybir
import concourse.tile as tile
from concourse import bacc
from concourse import bass_utils
from concourse.masks import make_identity

BF16 = mybir.dt.bfloat16
F32 = mybir.dt.float32
AF = mybir.ActivationFunctionType

B, S, E = 2, 2048, 1024
H, D = 16, 64
N_CORES = 8
HL = 4          # heads per core
DL = HL * D     # 256 local d
NPAIR = 2       # head pairs per core
KT_TILES = S // 128   # 16
QC = 4          # q chunks of 512
ET = E // 128   # 8 e-tiles


def _install_ntff_hook():
    """Register the axon NTFF profiling hook if the image's antenv lacks it."""
    try:
        import antenv  # noqa
        if 'antenv.axon_hooks' in sys.modules:
            return
        mod = types.ModuleType('antenv.axon_hooks')
        _hook = [None]
        mod.set_axon_ntff_profile_hook = lambda h: _hook.__setitem__(0, h)
        mod.get_axon_ntff_profile_hook = lambda: _hook[0]
        sys.modules['antenv.axon_hooks'] = mod
        setattr(antenv, 'axon_hooks', mod)
        try:
            from trn_agent_boot.trn_boot import _ntff_profile_via_ctypes
            h = _ntff_profile_via_ctypes('/opt/axon/libaxon_pjrt.so')
            if h is not None:
                mod.set_axon_ntff_profile_hook(h)
        except Exception:
            pass
    except Exception:
        pass


def build_kernel():
    nc = bacc.Bacc("TRN2", target_bir_lowering=False, debug=False,
                   enable_asserts=True, num_devices=N_CORES)

    # all inputs pre-arranged on host to be contiguous for their SBUF tiles
    xq_ap = nc.dram_tensor("xq_t", [QC, 128, ET, 512], BF16, kind="ExternalInput").ap()
    xk_ap = nc.dram_tensor("xk_t", [QC, 128, ET, 512], BF16, kind="ExternalInput").ap()
    xv_ap = nc.dram_tensor("xv_t", [QC, 128, ET, 512], BF16, kind="ExternalInput").ap()
    wq_ap = nc.dram_tensor("wq", [128, ET, DL], BF16, kind="ExternalInput").ap()
    wk_ap = nc.dram_tensor("wk", [128, ET, DL], BF16, kind="ExternalInput").ap()
    wv_ap = nc.dram_tensor("wv", [128, ET, HL * 65], BF16, kind="ExternalInput").ap()
    bq_ap = nc.dram_tensor("bq", [128, 2], F32, kind="ExternalInput").ap()
    bk_ap = nc.dram_tensor("bk", [128, 2], F32, kind="ExternalInput").ap()
    bv_ap = nc.dram_tensor("bv", [1, HL * 65], BF16, kind="ExternalInput").ap()
    wo_ap = nc.dram_tensor("wo", [128, 2, E], BF16, kind="ExternalInput").ap()
    out_ap = nc.dram_tensor("out_p", [S, E], F32, kind="ExternalOutput").ap()

    from contextlib import ExitStack
    with tile.TileContext(nc) as tc, ExitStack() as ctx:
        wpool = ctx.enter_context(tc.tile_pool(name="w", bufs=1))
        xtp = ctx.enter_context(tc.tile_pool(name="xt", bufs=5))
        big = ctx.enter_context(tc.tile_pool(name="big", bufs=1))
        expp = ctx.enter_context(tc.tile_pool(name="expp", bufs=4))
        znp = ctx.enter_context(tc.tile_pool(name="znp", bufs=2))
        smal = ctx.enter_context(tc.tile_pool(name="small", bufs=2))
        stg = ctx.enter_context(tc.tile_pool(name="stg", bufs=1))
        pscore = ctx.enter_context(tc.tile_pool(name="pscore", bufs=2, space="PSUM"))
        pav = ctx.enter_context(tc.tile_pool(name="pav", bufs=1, space="PSUM"))
        ptr = ctx.enter_context(tc.tile_pool(name="ptr", bufs=1, space="PSUM"))
        ppo = ctx.enter_context(tc.tile_pool(name="ppo", bufs=2, space="PSUM"))

        # ---- persistent weights / constants ----
        wq_sb = wpool.tile([128, ET, DL], BF16, tag="wq")
        wk_sb = wpool.tile([128, ET, DL], BF16, tag="wk")
        wv_sb = wpool.tile([128, ET, HL * 65], BF16, tag="wv")
        wo_sb = wpool.tile([128, 2, E], BF16, tag="wo")
        bq_sb = wpool.tile([128, 2], F32, tag="bq")
        bk_sb = wpool.tile([128, 2], F32, tag="bk")
        bv_sb = wpool.tile([1, HL * 65], BF16, tag="bv")
        ones_col = wpool.tile([1, 128], BF16, tag="ones")
        ident = wpool.tile([128, 128], BF16, tag="ident")

        nc.vector.memset(ones_col[:], 1.0)
        make_identity(nc, ident[:])

        QT = big.tile([128, NPAIR, S], BF16, tag="QT")
        KT = big.tile([128, NPAIR, S], BF16, tag="KT")
        Vones = big.tile([128, KT_TILES, HL, 65], BF16, tag="Vones")
        ZT = big.tile([128, NPAIR, S], BF16, tag="ZT")

        def load_xt_half(ap, sc, hf):
            # half of a 512-column slice of X^T: [128, 4 e-tiles, 512]
            t = xtp.tile([128, ET // 2, 512], BF16, tag="xt", name="xt")
            nc.sync.dma_start(t[:], ap[sc][:, hf * 4:(hf + 1) * 4, :])
            return t

        def load_xt_sc(ap, sc):
            return (load_xt_half(ap, sc, 0), load_xt_half(ap, sc, 1))

        def xe(x_pair, e):
            return x_pair[e // 4][:, e % 4, :]

        def proj_qk_sc(dst, w_sb, b_sb, x_sc, p, sc):
            # dst[:, p, sc-block] (transposed proj): out[d(128), s] = W.T @ X^T
            ps = ppo.tile([128, 512], F32, tag="ppo")
            for e in range(ET):
                nc.tensor.matmul(
                    ps[:], w_sb[:, e, p * 128:(p + 1) * 128], xe(x_sc, e),
                    start=(e == 0), stop=(e == ET - 1))
            nc.vector.tensor_scalar_add(
                dst[:, p, sc * 512:(sc + 1) * 512], ps[:], b_sb[:, p:p + 1])

        def gen_projqk(dst, w_sb, b_sb, x_ap, p):
            # generator: ~2 matmuls (0.45us) per pull
            for sc in range(QC):
                x_sc = load_xt_sc(x_ap, sc)
                ps = ppo.tile([128, 512], F32, tag="ppo", name="ps")
                for e in range(ET):
                    nc.tensor.matmul(
                        ps[:], w_sb[:, e, p * 128:(p + 1) * 128], xe(x_sc, e),
                        start=(e == 0), stop=(e == ET - 1))
                    if e % 2 == 1:
                        yield
                nc.vector.tensor_scalar_add(
                    dst[:, p, sc * 512:(sc + 1) * 512], ps[:], b_sb[:, p:p + 1])
                yield

        def gen_projv(x_ap):
            # single-pass V projection (all 4 heads, N=260), smeared
            for vsc in range(QC):
                x_sc = load_xt_sc(x_ap, vsc)
                for sti in range(4):
                    st = vsc * 4 + sti
                    ps = ppo.tile([128, HL * 65], F32, tag="ppo", name="ps")
                    for e in range(ET):
                        nc.tensor.matmul(
                            ps[:], xe(x_sc, e)[:, sti * 128:(sti + 1) * 128],
                            wv_sb[:, e, :],
                            start=(e == 0), stop=False)
                        if e % 2 == 1:
                            yield
                    nc.tensor.matmul(ps[:], ones_col[:], bv_sb[:],
                                     start=False, stop=True)
                    nc.vector.tensor_copy(
                        Vones[:, st], ps[:].rearrange("p (h d) -> p h d", h=HL))
                    yield

        def gen_av(p, qc, et):
            # AV + normalize + transpose for one (pair, q-chunk); ~0.3us per pull
            zn = znp.tile([128, 4, 2, D], BF16, tag="zn", name="zn")
            for h in range(2):
                avp = pav.tile([128, 4, 65], F32, tag="av", name="avp")
                # qt-outer: interleaved accumulation groups in one PSUM bank
                # are NOT allowed (each group's start clears the whole bank's
                # has_written bits) — a qt group must fully precede the next.
                for qt in range(4):
                    for kt in range(KT_TILES):
                        nc.tensor.matmul(
                            avp[:, qt, :],
                            et[:, kt, h, qt * 128:(qt + 1) * 128],
                            Vones[:, kt, 2 * p + h, :],
                            start=(kt == 0), stop=(kt == KT_TILES - 1))
                        if kt % 8 == 7:
                            yield
                rc = smal.tile([128, 4, 1], F32, tag="rc", name="rc")
                nc.vector.reciprocal(rc[:], avp[:, :, 64:65])
                nc.vector.tensor_mul(zn[:, :, h, :], avp[:, :, 0:D],
                                     rc[:].to_broadcast([128, 4, D]))
                yield
            for qt in range(4):
                tp = ptr.tile([128, 128], BF16, tag="tr", name="tp")
                nc.tensor.transpose(tp[:], zn[:, qt], ident[:])
                nc.vector.tensor_copy(
                    ZT[:, p, qc * 512 + qt * 128: qc * 512 + (qt + 1) * 128], tp[:])
                if qt % 2 == 1:
                    yield

        def gen_outproj(sts, act_evict=False):
            for st in sts:
                stt = stg.tile([128, 2, 512], F32, tag="stg", name="stt")
                for ec in range(2):
                    ps = ppo.tile([128, 512], F32, tag="ppo", name="ps")
                    for dt_ in range(2):
                        nc.tensor.matmul(
                            ps[:], ZT[:, dt_, st * 128:(st + 1) * 128],
                            wo_sb[:, dt_, ec * 512:(ec + 1) * 512],
                            start=(dt_ == 0), stop=(dt_ == 1))
                    if act_evict and ec == 1:
                        nc.scalar.copy(stt[:, ec], ps[:])
                    else:
                        nc.vector.tensor_copy(stt[:, ec], ps[:])
                    nc.sync.dma_start(
                        out_ap[st * 128:(st + 1) * 128, ec * 512:(ec + 1) * 512],
                        stt[:, ec])
                    yield

        def scores_kts(p, qc, et, kts):
            for kt in kts:
                sc_t = pscore.tile([128, 2, 512], F32, tag="sc")
                for h in range(2):
                    nc.tensor.matmul(
                        sc_t[:, h, :],
                        KT[64 * h:64 * (h + 1), p, kt * 128:(kt + 1) * 128],
                        QT[64 * h:64 * (h + 1), p, qc * 512:(qc + 1) * 512],
                        start=True, stop=True, tile_position=(64 * h, 0))
                nc.scalar.activation(et[:, kt], sc_t[:], AF.Exp)

        # ---- emission (static per-engine order ~ schedule priority) ----
        # PE warmup (HAM): dummy matmuls on a zeroed tile during the DMA lead-in
        warm = wpool.tile([128, 512], BF16, tag="warm")
        nc.vector.memset(warm[:], 0.0)
        wps = ppo.tile([128, 512], F32, tag="ppo")
        for i in range(10):
            nc.tensor.matmul(wps[:], warm[:, 0:128], warm[:],
                             start=(i == 0), stop=(i == 9))

        # DMA order: xq-sc0, wq, xk-sc0, wk first so scores (-> exp) start ASAP
        nc.sync.dma_start(wq_sb[:], wq_ap[:])
        nc.sync.dma_start(wk_sb[:], wk_ap[:])

        # first q-chunk of scores interleaved with the QK projections
        et00 = expp.tile([128, KT_TILES, 2, 512], BF16, tag="expT")
        for sc in range(QC):
            xq_sc = load_xt_sc(xq_ap, sc)
            xk_sc = load_xt_sc(xk_ap, sc)
            if sc == 0:
                nc.sync.dma_start(bq_sb[:], bq_ap[:])
                nc.sync.dma_start(bk_sb[:], bk_ap[:])
            proj_qk_sc(QT, wq_sb, bq_sb, xq_sc, 0, sc)
            proj_qk_sc(KT, wk_sb, bk_sb, xk_sc, 0, sc)
            scores_kts(0, 0, et00, range(4 * sc, 4 * sc + 4))

        # V path loads (needed by first av phase)
        nc.sync.dma_start(wv_sb[:], wv_ap[:])
        nc.sync.dma_start(bv_sb[:], bv_ap[:])
        nc.sync.dma_start(wo_sb[:], wo_ap[:])

        def new_et():
            return expp.tile([128, KT_TILES, 2, 512], BF16, tag="expT", name="et")

        def drain(g, n=10 ** 9):
            """Pull generator g up to n times; True if exhausted."""
            for _ in range(n):
                if next(g, StopIteration) is StopIteration:
                    return True
            return False

        # background work generators, smeared between scores kt's (FIFO so only
        # one AV psum tile is live at a time)
        gv = gen_projv(xv_ap)
        gqt1 = gen_projqk(QT, wq_sb, bq_sb, xq_ap, 1)
        gkt1 = gen_projqk(KT, wk_sb, bk_sb, xk_ap, 1)
        ets = {(0, 0): et00}

        def run_loop(p, qc, fifo, budget):
            """Emit scores(p, qc) kt-by-kt, pulling `budget` steps per kt from
            the FIFO of filler generators."""
            et = ets.setdefault((p, qc), new_et())
            for kt in range(KT_TILES):
                scores_kts(p, qc, et, [kt])
                left = budget
                while left > 0 and fifo:
                    if drain(fifo[0], left):
                        fifo.pop(0)
                        left -= 1  # approximate
                    else:
                        left = 0

        # block-style emission: scores 4-kt blocks alternating with background
        # work blocks (~one vsc/sc group at a time)
        # loop 0: scores(0,1) + first half of V projection (2-kt granularity)
        et01 = ets.setdefault((0, 1), new_et())
        for i in range(8):
            scores_kts(0, 1, et01, range(2 * i, 2 * i + 2))
            drain(gv, 5)
        # loop 1: scores(0,2) + rest of V
        et02 = ets.setdefault((0, 2), new_et())
        for i in range(8):
            scores_kts(0, 2, et02, range(2 * i, 2 * i + 2))
            drain(gv, 5)
        drain(gv)
        # loop 2: scores(0,3) + QT1 + av(0,0)
        ga00 = gen_av(0, 0, ets[(0, 0)])
        et03 = ets.setdefault((0, 3), new_et())
        for i in range(8):
            scores_kts(0, 3, et03, range(2 * i, 2 * i + 2))
            drain(gqt1, 3)
            drain(ga00, 3)
        drain(gqt1)
        drain(ga00)
        # loop 3: scores(1,0) + KT1 (sc-block ordered ahead) + av(0,1)
        ga01 = gen_av(0, 1, ets[(0, 1)])
        et10 = ets.setdefault((1, 0), new_et())
        for sc in range(QC):
            drain(gkt1, 6)  # one full sc block of KT1
            scores_kts(1, 0, et10, range(4 * sc, 4 * sc + 2))
            drain(ga01, 3)
            scores_kts(1, 0, et10, range(4 * sc + 2, 4 * sc + 4))
            drain(ga01, 3)
        drain(gkt1)
        drain(ga01)
        # loop 4: scores(1,1) + av(1,0) + av(0,2)
        ga10 = gen_av(1, 0, ets[(1, 0)])
        ga02 = gen_av(0, 2, ets[(0, 2)])
        et11 = ets.setdefault((1, 1), new_et())
        for i in range(8):
            scores_kts(1, 1, et11, range(2 * i, 2 * i + 2))
            drain(ga10, 3)
            drain(ga02, 3)
        drain(ga10)
        drain(ga02)
        # loop 5: scores(1,2) + av(1,1) + av(0,3) + outproj(0-3)
        ga11 = gen_av(1, 1, ets[(1, 1)])
        ga03 = gen_av(0, 3, ets[(0, 3)])
        gop0 = gen_outproj(range(0, 4))
        et12 = ets.setdefault((1, 2), new_et())
        for i in range(8):
            scores_kts(1, 2, et12, range(2 * i, 2 * i + 2))
            drain(ga11, 3)
            drain(ga03, 3)
            drain(gop0, 1)
        drain(ga11)
        drain(ga03)
        # loop 6: scores(1,3) + av(1,2) + outproj(4-11)
        ga12 = gen_av(1, 2, ets[(1, 2)])
        gop1 = gen_outproj(range(4, 8))
        gop2 = gen_outproj(range(8, 12))
        et13 = ets.setdefault((1, 3), new_et())
        for i in range(8):
            scores_kts(1, 3, et13, range(2 * i, 2 * i + 2))
            drain(gop0, 1)
            drain(ga12, 3)
            drain(gop1, 1)
        drain(gop0)
        drain(ga12)
        drain(gop1)
        # tail: av(1,3) + outproj(8-15)
        ga13 = gen_av(1, 3, ets[(1, 3)])
        drain(ga13)
        gop3 = gen_outproj(range(12, 16), act_evict=True)
        drain(gop2)
        drain(gop3)

    nc.compile()
    return nc


def prep_inputs(query, key, value, Wq, bq, Wk, bk, Wv, bv, Wo, bo):
    """Host-side sharding: per-core input dicts (bf16, transposed/augmented)."""
    bf = ml_dtypes.bfloat16
    q32 = np.asarray(query, np.float32)
    k32 = np.asarray(key, np.float32)
    v32 = np.asarray(value, np.float32)
    Wq = np.asarray(Wq, np.float32)
    Wk = np.asarray(Wk, np.float32)
    Wv = np.asarray(Wv, np.float32)
    Wo = np.asarray(Wo, np.float32)
    bq = np.asarray(bq, np.float32)
    bk = np.asarray(bk, np.float32)
    bv = np.asarray(bv, np.float32)

    scale = 1.0 / np.sqrt(np.float32(D))

    def xt_layout(x2d):
        # [S, E] -> X^T [E, S] -> [sc, p, eo, j] contiguous tile layout
        a = x2d.T.reshape(ET, 128, QC, 512).transpose(2, 1, 0, 3)
        return np.ascontiguousarray(a).astype(bf)

    def w_layout(w2d):
        # [E, D'] -> [p, eo, D'] contiguous
        a = w2d.reshape(ET, 128, w2d.shape[1]).transpose(1, 0, 2)
        return np.ascontiguousarray(a).astype(bf)

    xt = {}
    for b in range(B):
        xt[('q', b)] = xt_layout(q32[b])
        xt[('k', b)] = xt_layout(k32[b])
        xt[('v', b)] = xt_layout(v32[b])

    in_maps = []
    for c in range(N_CORES):
        b, g = c // HL, c % HL
        hs = slice(g * DL, (g + 1) * DL)
        wv_aug = np.zeros((E, HL * 65), np.float32)
        bv_aug = np.zeros((1, HL * 65), np.float32)
        for h in range(HL):
            wv_aug[:, h * 65:h * 65 + D] = Wv[:, g * DL + h * D: g * DL + (h + 1) * D]
            bv_aug[0, h * 65:h * 65 + D] = bv[g * DL + h * D: g * DL + (h + 1) * D]
            bv_aug[0, h * 65 + D] = 1.0
        in_maps.append({
            "xq_t": xt[('q', b)],
            "xk_t": xt[('k', b)],
            "xv_t": xt[('v', b)],
            "wq": w_layout(Wq[:, hs] * scale),
            "wk": w_layout(Wk[:, hs]),
            "wv": w_layout(wv_aug),
            "bq": np.ascontiguousarray(
                (bq[hs] * scale).reshape(2, 128).T).astype(np.float32),
            "bk": np.ascontiguousarray(
                bk[hs].reshape(2, 128).T).astype(np.float32),
            "bv": bv_aug.astype(bf),
            "wo": np.ascontiguousarray(
                Wo[hs, :].reshape(2, 128, E).transpose(1, 0, 2)).astype(bf),
        })
    return in_maps


_NC_CACHE = [None]


def get_nc():
    if _NC_CACHE[0] is None:
        _install_ntff_hook()
        _NC_CACHE[0] = build_kernel()
    return _NC_CACHE[0]


def run(inputs, trace=False):
    nc = get_nc()
    in_maps = prep_inputs(**{k: v for k, v in inputs.items() if k != 'bo'},
                          bo=inputs['bo'])
    res = bass_utils.run_bass_kernel_spmd(
        nc, in_maps, core_ids=list(range(N_CORES)), trace=trace)
    bo = np.asarray(inputs['bo'], np.float32)
    out = np.empty((B, S, E), np.float32)
    for b in range(B):
        acc = np.zeros((S, E), np.float32)
        for g in range(HL):
            acc += res.results[b * HL + g]["out_p"]
        out[b] = acc + bo[None, :]
    return out, res


def kernel(**inputs):
    out, _ = run(inputs, trace=False)
    return out


# revision 43
# speedup vs baseline: 1.0541x; 1.0541x over previous
"""Trainium2 Bass kernel for nn_MultiHeadAttention (B=2, S=2048, E=1024, H=16, D=64).

Sharding: 8 cores = 2 batches x 4 head-groups (4 heads / core, d_local=256).
Each core computes, for its (batch b, head group g):
    q = Xq[b] @ Wq[:, hs]*0.125 + bq[hs]*0.125        (transposed layout QT [256, S])
    k = Xk[b] @ Wk[:, hs] + bk[hs]                    (transposed layout KT [256, S])
    v = Xv[b] @ Wv[:, hs] + bv[hs]                    (natural layout, 65-strided + ones col)
    per head: scores^T = K_h @ Q_h^T  -> exp (ACT) -> Z|denom = expW^T.T @ [V_h|1]
    Z normalized per-partition, PE-transposed to ZT [256, S]
    partial_out = Z @ Wo[hs, :]                       ([S, E] fp32, host sums over g)
Host: transposes/casts inputs to bf16, sums the 4 partials per batch, adds bo.

Self-contained: hardcodes all shapes; requires only concourse (+ml_dtypes/numpy).
"""

import sys
import types

import numpy as np
import ml_dtypes

import concourse.bass as bass  # noqa: F401  (bass types used via tile/bacc)
import concourse.mybir as mybir
import concourse.tile as tile
from concourse import bacc
from concourse import bass_utils
from concourse.masks import make_identity

BF16 = mybir.dt.bfloat16
F32 = mybir.dt.float32
AF = mybir.ActivationFunctionType

B, S, E = 2, 2048, 1024
H, D = 16, 64
N_CORES = 8
HL = 4          # heads per core
DL = HL * D     # 256 local d
NPAIR = 2       # head pairs per core
KT_TILES = S // 128   # 16
QC = 4          # q chunks of 512
ET = E // 128   # 8 e-tiles


def _install_ntff_hook():
    """Register the axon NTFF profiling hook if the image's antenv lacks it."""
    try:
        import antenv  # noqa
        if 'antenv.axon_hooks' in sys.modules:
            return
        mod = types.ModuleType('antenv.axon_hooks')
        _hook = [None]
        mod.set_axon_ntff_profile_hook = lambda h: _hook.__setitem__(0, h)
        mod.get_axon_ntff_profile_hook = lambda: _hook[0]
        sys.modules['antenv.axon_hooks'] = mod
        setattr(antenv, 'axon_hooks', mod)
        try:
            from trn_agent_boot.trn_boot import _ntff_profile_via_ctypes
            h = _ntff_profile_via_ctypes('/opt/axon/libaxon_pjrt.so')
            if h is not None:
                mod.set_axon_ntff_profile_hook(h)
        except Exception:
            pass
    except Exception:
        pass


def build_kernel():
    nc = bacc.Bacc("TRN2", target_bir_lowering=False, debug=False,
                   enable_asserts=True, num_devices=N_CORES)

    # all inputs pre-arranged on host to be contiguous for their SBUF tiles
    xq_ap = nc.dram_tensor("xq_t", [QC, 128, ET, 512], BF16, kind="ExternalInput").ap()
    xk_ap = nc.dram_tensor("xk_t", [QC, 128, ET, 512], BF16, kind="ExternalInput").ap()
    xv_ap = nc.dram_tensor("xv_t", [QC, 128, ET, 512], BF16, kind="ExternalInput").ap()
    wq_ap = nc.dram_tensor("wq", [128, ET, DL], BF16, kind="ExternalInput").ap()
    wk_ap = nc.dram_tensor("wk", [128, ET, DL], BF16, kind="ExternalInput").ap()
    wv_ap = nc.dram_tensor("wv", [128, ET, HL * 65], BF16, kind="ExternalInput").ap()
    bq_ap = nc.dram_tensor("bq", [128, 2], F32, kind="ExternalInput").ap()
    bk_ap = nc.dram_tensor("bk", [128, 2], F32, kind="ExternalInput").ap()
    bv_ap = nc.dram_tensor("bv", [1, HL * 65], BF16, kind="ExternalInput").ap()
    wo_ap = nc.dram_tensor("wo", [128, 2, E], BF16, kind="ExternalInput").ap()
    out_ap = nc.dram_tensor("out_p", [S, E], F32, kind="ExternalOutput").ap()

    from contextlib import ExitStack
    with tile.TileContext(nc) as tc, ExitStack() as ctx:
        wpool = ctx.enter_context(tc.tile_pool(name="w", bufs=1))
        xtp = ctx.enter_context(tc.tile_pool(name="xt", bufs=5))
        big = ctx.enter_context(tc.tile_pool(name="big", bufs=1))
        expp = ctx.enter_context(tc.tile_pool(name="expp", bufs=4))
        znp = ctx.enter_context(tc.tile_pool(name="znp", bufs=2))
        smal = ctx.enter_context(tc.tile_pool(name="small", bufs=2))
        stg = ctx.enter_context(tc.tile_pool(name="stg", bufs=1))
        pscore = ctx.enter_context(tc.tile_pool(name="pscore", bufs=2, space="PSUM"))
        pav = ctx.enter_context(tc.tile_pool(name="pav", bufs=1, space="PSUM"))
        ptr = ctx.enter_context(tc.tile_pool(name="ptr", bufs=1, space="PSUM"))
        ppo = ctx.enter_context(tc.tile_pool(name="ppo", bufs=2, space="PSUM"))

        # ---- persistent weights / constants ----
        wq_sb = wpool.tile([128, ET, DL], BF16, tag="wq")
        wk_sb = wpool.tile([128, ET, DL], BF16, tag="wk")
        wv_sb = wpool.tile([128, ET, HL * 65], BF16, tag="wv")
        wo_sb = wpool.tile([128, 2, E], BF16, tag="wo")
        bq_sb = wpool.tile([128, 2], F32, tag="bq")
        bk_sb = wpool.tile([128, 2], F32, tag="bk")
        bv_sb = wpool.tile([1, HL * 65], BF16, tag="bv")
        ones_col = wpool.tile([1, 128], BF16, tag="ones")
        ident = wpool.tile([128, 128], BF16, tag="ident")

        nc.vector.memset(ones_col[:], 1.0)
        make_identity(nc, ident[:])

        QT = big.tile([128, NPAIR, S], BF16, tag="QT")
        KT = big.tile([128, NPAIR, S], BF16, tag="KT")
        Vones = big.tile([128, KT_TILES, HL, 65], BF16, tag="Vones")
        ZT = big.tile([128, NPAIR, S], BF16, tag="ZT")

        def load_xt_half(ap, sc, hf):
            # half of a 512-column slice of X^T: [128, 4 e-tiles, 512]
            t = xtp.tile([128, ET // 2, 512], BF16, tag="xt", name="xt")
            nc.sync.dma_start(t[:], ap[sc][:, hf * 4:(hf + 1) * 4, :])
            return t

        def load_xt_sc(ap, sc):
            return (load_xt_half(ap, sc, 0), load_xt_half(ap, sc, 1))

        def xe(x_pair, e):
            return x_pair[e // 4][:, e % 4, :]

        def proj_qk_sc(dst, w_sb, b_sb, x_sc, p, sc):
            # dst[:, p, sc-block] (transposed proj): out[d(128), s] = W.T @ X^T
            ps = ppo.tile([128, 512], F32, tag="ppo")
            for e in range(ET):
                nc.tensor.matmul(
                    ps[:], w_sb[:, e, p * 128:(p + 1) * 128], xe(x_sc, e),
                    start=(e == 0), stop=(e == ET - 1))
            nc.vector.tensor_scalar_add(
                dst[:, p, sc * 512:(sc + 1) * 512], ps[:], b_sb[:, p:p + 1])

        def gen_projqk(dst, w_sb, b_sb, x_ap, p):
            # generator: ~2 matmuls (0.45us) per pull
            for sc in range(QC):
                x_sc = load_xt_sc(x_ap, sc)
                ps = ppo.tile([128, 512], F32, tag="ppo", name="ps")
                for e in range(ET):
                    nc.tensor.matmul(
                        ps[:], w_sb[:, e, p * 128:(p + 1) * 128], xe(x_sc, e),
                        start=(e == 0), stop=(e == ET - 1))
                    if e % 2 == 1:
                        yield
                nc.vector.tensor_scalar_add(
                    dst[:, p, sc * 512:(sc + 1) * 512], ps[:], b_sb[:, p:p + 1])
                yield

        def gen_projv(x_ap):
            # single-pass V projection (all 4 heads, N=260), smeared
            for vsc in range(QC):
                x_sc = load_xt_sc(x_ap, vsc)
                for sti in range(4):
                    st = vsc * 4 + sti
                    ps = ppo.tile([128, HL * 65], F32, tag="ppo", name="ps")
                    for e in range(ET):
                        nc.tensor.matmul(
                            ps[:], xe(x_sc, e)[:, sti * 128:(sti + 1) * 128],
                            wv_sb[:, e, :],
                            start=(e == 0), stop=False)
                        if e % 2 == 1:
                            yield
                    nc.tensor.matmul(ps[:], ones_col[:], bv_sb[:],
                                     start=False, stop=True)
                    nc.vector.tensor_copy(
                        Vones[:, st], ps[:].rearrange("p (h d) -> p h d", h=HL))
                    yield

        def gen_av(p, qc, et):
            # AV + normalize + transpose for one (pair, q-chunk); ~0.3us per pull
            zn = znp.tile([128, 4, 2, D], BF16, tag="zn", name="zn")
            for h in range(2):
                avp = pav.tile([128, 4, 65], F32, tag="av", name="avp")
                # qt-outer: interleaved accumulation groups in one PSUM bank
                # are NOT allowed (each group's start clears the whole bank's
                # has_written bits) — a qt group must fully precede the next.
                for qt in range(4):
                    for kt in range(KT_TILES):
                        nc.tensor.matmul(
                            avp[:, qt, :],
                            et[:, kt, h, qt * 128:(qt + 1) * 128],
                            Vones[:, kt, 2 * p + h, :],
                            start=(kt == 0), stop=(kt == KT_TILES - 1))
                        if kt % 8 == 7:
                            yield
                rc = smal.tile([128, 4, 1], F32, tag="rc", name="rc")
                nc.vector.reciprocal(rc[:], avp[:, :, 64:65])
                nc.vector.tensor_mul(zn[:, :, h, :], avp[:, :, 0:D],
                                     rc[:].to_broadcast([128, 4, D]))
                yield
            for qt in range(4):
                tp = ptr.tile([128, 128], BF16, tag="tr", name="tp")
                nc.tensor.transpose(tp[:], zn[:, qt], ident[:])
                nc.vector.tensor_copy(
                    ZT[:, p, qc * 512 + qt * 128: qc * 512 + (qt + 1) * 128], tp[:])
                if qt % 2 == 1:
                    yield

        def gen_outproj(sts, act_evict=False):
            for st in sts:
                stt = stg.tile([128, 2, 512], F32, tag="stg", name="stt")
                for ec in range(2):
                    ps = ppo.tile([128, 512], F32, tag="ppo", name="ps")
                    for dt_ in range(2):
                        nc.tensor.matmul(
                            ps[:], ZT[:, dt_, st * 128:(st + 1) * 128],
                            wo_sb[:, dt_, ec * 512:(ec + 1) * 512],
                            start=(dt_ == 0), stop=(dt_ == 1))
                    if act_evict and ec == 1:
                        nc.scalar.copy(stt[:, ec], ps[:])
                    else:
                        nc.vector.tensor_copy(stt[:, ec], ps[:])
                    nc.sync.dma_start(
                        out_ap[st * 128:(st + 1) * 128, ec * 512:(ec + 1) * 512],
                        stt[:, ec])
                    yield

        def scores_kts(p, qc, et, kts):
            for kt in kts:
                sc_t = pscore.tile([128, 2, 512], F32, tag="sc")
                for h in range(2):
                    nc.tensor.matmul(
                        sc_t[:, h, :],
                        KT[64 * h:64 * (h + 1), p, kt * 128:(kt + 1) * 128],
                        QT[64 * h:64 * (h + 1), p, qc * 512:(qc + 1) * 512],
                        start=True, stop=True, tile_position=(64 * h, 0))
                nc.scalar.activation(et[:, kt], sc_t[:], AF.Exp)

        # ---- emission (static per-engine order ~ schedule priority) ----
        # PE warmup (HAM): dummy matmuls on a zeroed tile during the DMA lead-in
        warm = wpool.tile([128, 512], BF16, tag="warm")
        nc.vector.memset(warm[:], 0.0)
        wps = ppo.tile([128, 512], F32, tag="ppo")
        for i in range(10):
            nc.tensor.matmul(wps[:], warm[:, 0:128], warm[:],
                             start=(i == 0), stop=(i == 9))

        # DMA order: xq-sc0, wq, xk-sc0, wk first so scores (-> exp) start ASAP
        nc.sync.dma_start(wq_sb[:], wq_ap[:])
        nc.sync.dma_start(wk_sb[:], wk_ap[:])

        # first q-chunk of scores interleaved with the QK projections
        et00 = expp.tile([128, KT_TILES, 2, 512], BF16, tag="expT")
        for sc in range(QC):
            xq_sc = load_xt_sc(xq_ap, sc)
            xk_sc = load_xt_sc(xk_ap, sc)
            if sc == 0:
                nc.sync.dma_start(bq_sb[:], bq_ap[:])
                nc.sync.dma_start(bk_sb[:], bk_ap[:])
            proj_qk_sc(QT, wq_sb, bq_sb, xq_sc, 0, sc)
            proj_qk_sc(KT, wk_sb, bk_sb, xk_sc, 0, sc)
            scores_kts(0, 0, et00, range(4 * sc, 4 * sc + 4))

        # V path loads (needed by first av phase)
        nc.sync.dma_start(wv_sb[:], wv_ap[:])
        nc.sync.dma_start(bv_sb[:], bv_ap[:])
        nc.sync.dma_start(wo_sb[:], wo_ap[:])

        def new_et():
            return expp.tile([128, KT_TILES, 2, 512], BF16, tag="expT", name="et")

        def drain(g, n=10 ** 9):
            """Pull generator g up to n times; True if exhausted."""
            for _ in range(n):
                if next(g, StopIteration) is StopIteration:
                    return True
            return False

        # background work generators, smeared between scores kt's (FIFO so only
        # one AV psum tile is live at a time)
        gv = gen_projv(xv_ap)
        gqt1 = gen_projqk(QT, wq_sb, bq_sb, xq_ap, 1)
        gkt1 = gen_projqk(KT, wk_sb, bk_sb, xk_ap, 1)
        ets = {(0, 0): et00}

        def run_loop(p, qc, fifo, budget):
            """Emit scores(p, qc) kt-by-kt, pulling `budget` steps per kt from
            the FIFO of filler generators."""
            et = ets.setdefault((p, qc), new_et())
            for kt in range(KT_TILES):
                scores_kts(p, qc, et, [kt])
                left = budget
                while left > 0 and fifo:
                    if drain(fifo[0], left):
                        fifo.pop(0)
                        left -= 1  # approximate
                    else:
                        left = 0

        # block-style emission: scores 4-kt blocks alternating with background
        # work blocks (~one vsc/sc group at a time)
        # loop 0: scores(0,1) + first half of V projection (2-kt granularity)
        et01 = ets.setdefault((0, 1), new_et())
        for i in range(8):
            scores_kts(0, 1, et01, range(2 * i, 2 * i + 2))
            drain(gv, 5)
        # loop 1: scores(0,2) + rest of V
        et02 = ets.setdefault((0, 2), new_et())
        for i in range(8):
            scores_kts(0, 2, et02, range(2 * i, 2 * i + 2))
            drain(gv, 5)
        drain(gv)
        # loop 2: scores(0,3) + QT1 + av(0,0)
        ga00 = gen_av(0, 0, ets[(0, 0)])
        et03 = ets.setdefault((0, 3), new_et())
        for i in range(8):
            scores_kts(0, 3, et03, range(2 * i, 2 * i + 2))
            drain(gqt1, 3)
            drain(ga00, 3)
        drain(gqt1)
        drain(ga00)
        # loop 3: scores(1,0) + KT1 (sc-block ordered ahead) + av(0,1)
        ga01 = gen_av(0, 1, ets[(0, 1)])
        et10 = ets.setdefault((1, 0), new_et())
        for sc in range(QC):
            drain(gkt1, 6)  # one full sc block of KT1
            scores_kts(1, 0, et10, range(4 * sc, 4 * sc + 2))
            drain(ga01, 3)
            scores_kts(1, 0, et10, range(4 * sc + 2, 4 * sc + 4))
            drain(ga01, 3)
        drain(gkt1)
        drain(ga01)
        # loop 4: scores(1,1) + av(1,0) + av(0,2)
        ga10 = gen_av(1, 0, ets[(1, 0)])
        ga02 = gen_av(0, 2, ets[(0, 2)])
        et11 = ets.setdefault((1, 1), new_et())
        for i in range(8):
            scores_kts(1, 1, et11, range(2 * i, 2 * i + 2))
            drain(ga10, 3)
            drain(ga02, 3)
        drain(ga10)
        drain(ga02)
        # loop 5: scores(1,2) + av(1,1) + av(0,3) + outproj(0-3)
        ga11 = gen_av(1, 1, ets[(1, 1)])
        ga03 = gen_av(0, 3, ets[(0, 3)])
        gop0 = gen_outproj(range(0, 4))
        et12 = ets.setdefault((1, 2), new_et())
        for i in range(8):
            scores_kts(1, 2, et12, range(2 * i, 2 * i + 2))
            drain(ga11, 3)
            drain(ga03, 3)
            drain(gop0, 1)
        drain(ga11)
        drain(ga03)
        # loop 6: scores(1,3) + av(1,2) + outproj(4-11)
        ga12 = gen_av(1, 2, ets[(1, 2)])
        gop1 = gen_outproj(range(4, 8))
        gop2 = gen_outproj(range(8, 12))
        et13 = ets.setdefault((1, 3), new_et())
        for i in range(8):
            scores_kts(1, 3, et13, range(2 * i, 2 * i + 2))
            drain(gop0, 1)
            drain(ga12, 3)
            drain(gop1, 1)
        drain(gop0)
        drain(ga12)
        drain(gop1)
        # tail: av(1,3) + outproj(8-15)
        ga13 = gen_av(1, 3, ets[(1, 3)])
        drain(ga13)
        gop3 = gen_outproj(range(12, 16), act_evict=True)
        drain(gop2)
        drain(gop3)

    nc.compile()
    return nc


def prep_inputs(query, key, value, Wq, bq, Wk, bk, Wv, bv, Wo, bo):
    """Host-side sharding: per-core input dicts (bf16, transposed/augmented)."""
    bf = ml_dtypes.bfloat16
    q32 = np.asarray(query, np.float32)
    k32 = np.asarray(key, np.float32)
    v32 = np.asarray(value, np.float32)
    Wq = np.asarray(Wq, np.float32)
    Wk = np.asarray(Wk, np.float32)
    Wv = np.asarray(Wv, np.float32)
    Wo = np.asarray(Wo, np.float32)
    bq = np.asarray(bq, np.float32)
    bk = np.asarray(bk, np.float32)
    bv = np.asarray(bv, np.float32)

    scale = 1.0 / np.sqrt(np.float32(D))

    def xt_layout(x2d):
        # [S, E] -> X^T [E, S] -> [sc, p, eo, j] contiguous tile layout
        a = x2d.T.reshape(ET, 128, QC, 512).transpose(2, 1, 0, 3)
        return np.ascontiguousarray(a).astype(bf)

    def w_layout(w2d):
        # [E, D'] -> [p, eo, D'] contiguous
        a = w2d.reshape(ET, 128, w2d.shape[1]).transpose(1, 0, 2)
        return np.ascontiguousarray(a).astype(bf)

    xt = {}
    for b in range(B):
        xt[('q', b)] = xt_layout(q32[b])
        xt[('k', b)] = xt_layout(k32[b])
        xt[('v', b)] = xt_layout(v32[b])

    in_maps = []
    for c in range(N_CORES):
        b, g = c // HL, c % HL
        hs = slice(g * DL, (g + 1) * DL)
        wv_aug = np.zeros((E, HL * 65), np.float32)
        bv_aug = np.zeros((1, HL * 65), np.float32)
        for h in range(HL):
            wv_aug[:, h * 65:h * 65 + D] = Wv[:, g * DL + h * D: g * DL + (h + 1) * D]
            bv_aug[0, h * 65:h * 65 + D] = bv[g * DL + h * D: g * DL + (h + 1) * D]
            bv_aug[0, h * 65 + D] = 1.0
        in_maps.append({
            "xq_t": xt[('q', b)],
            "xk_t": xt[('k', b)],
            "xv_t": xt[('v', b)],
            "wq": w_layout(Wq[:, hs] * scale),
            "wk": w_layout(Wk[:, hs]),
            "wv": w_layout(wv_aug),
            "bq": np.ascontiguousarray(
                (bq[hs] * scale).reshape(2, 128).T).astype(np.float32),
            "bk": np.ascontiguousarray(
                bk[hs].reshape(2, 128).T).astype(np.float32),
            "bv": bv_aug.astype(bf),
            "wo": np.ascontiguousarray(
                Wo[hs, :].reshape(2, 128, E).transpose(1, 0, 2)).astype(bf),
        })
    return in_maps


_NC_CACHE = [None]


def get_nc():
    if _NC_CACHE[0] is None:
        _install_ntff_hook()
        _NC_CACHE[0] = build_kernel()
    return _NC_CACHE[0]


def run(inputs, trace=False):
    nc = get_nc()
    in_maps = prep_inputs(**{k: v for k, v in inputs.items() if k != 'bo'},
                          bo=inputs['bo'])
    res = bass_utils.run_bass_kernel_spmd(
        nc, in_maps, core_ids=list(range(N_CORES)), trace=trace)
    bo = np.asarray(inputs['bo'], np.float32)
    out = np.empty((B, S, E), np.float32)
    for b in range(B):
        acc = np.zeros((S, E), np.float32)
        for g in range(HL):
            acc += res.results[b * HL + g]["out_p"]
        out[b] = acc + bo[None, :]
    return out, res


def kernel(**inputs):
    out, _ = run(inputs, trace=False)
    return out


# revision 45
# speedup vs baseline: 1.1012x; 1.0446x over previous
"""Trainium2 Bass kernel for nn_MultiHeadAttention (B=2, S=2048, E=1024, H=16, D=64).

Sharding: 8 cores = 2 batches x 4 head-groups (4 heads / core, d_local=256).
Each core computes, for its (batch b, head group g):
    q = Xq[b] @ Wq[:, hs]*0.125 + bq[hs]*0.125        (transposed layout QT [256, S])
    k = Xk[b] @ Wk[:, hs] + bk[hs]                    (transposed layout KT [256, S])
    v = Xv[b] @ Wv[:, hs] + bv[hs]                    (natural layout, 65-strided + ones col)
    per head: scores^T = K_h @ Q_h^T  -> exp (ACT) -> Z|denom = expW^T.T @ [V_h|1]
    Z normalized per-partition, PE-transposed to ZT [256, S]
    partial_out = Z @ Wo[hs, :]                       ([S, E] fp32, host sums over g)
Host: transposes/casts inputs to bf16, sums the 4 partials per batch, adds bo.

Self-contained: hardcodes all shapes; requires only concourse (+ml_dtypes/numpy).
"""

import sys
import types

import numpy as np
import ml_dtypes

import concourse.bass as bass  # noqa: F401  (bass types used via tile/bacc)
import concourse.mybir as mybir
import concourse.tile as tile
from concourse import bacc
from concourse import bass_utils
from concourse.masks import make_identity

BF16 = mybir.dt.bfloat16
F32 = mybir.dt.float32
AF = mybir.ActivationFunctionType

B, S, E = 2, 2048, 1024
H, D = 16, 64
N_CORES = 8
HL = 4          # heads per core
DL = HL * D     # 256 local d
NPAIR = 2       # head pairs per core
KT_TILES = S // 128   # 16
QC = 4          # q chunks of 512
ET = E // 128   # 8 e-tiles


def _install_ntff_hook():
    """Register the axon NTFF profiling hook if the image's antenv lacks it."""
    try:
        import antenv  # noqa
        if 'antenv.axon_hooks' in sys.modules:
            return
        mod = types.ModuleType('antenv.axon_hooks')
        _hook = [None]
        mod.set_axon_ntff_profile_hook = lambda h: _hook.__setitem__(0, h)
        mod.get_axon_ntff_profile_hook = lambda: _hook[0]
        sys.modules['antenv.axon_hooks'] = mod
        setattr(antenv, 'axon_hooks', mod)
        try:
            from trn_agent_boot.trn_boot import _ntff_profile_via_ctypes
            h = _ntff_profile_via_ctypes('/opt/axon/libaxon_pjrt.so')
            if h is not None:
                mod.set_axon_ntff_profile_hook(h)
        except Exception:
            pass
    except Exception:
        pass


def build_kernel():
    nc = bacc.Bacc("TRN2", target_bir_lowering=False, debug=False,
                   enable_asserts=True, num_devices=N_CORES)

    # all inputs pre-arranged on host to be contiguous for their SBUF tiles
    xq_ap = nc.dram_tensor("xq_t", [QC, 128, ET, 512], BF16, kind="ExternalInput").ap()
    xk_ap = nc.dram_tensor("xk_t", [QC, 128, ET, 512], BF16, kind="ExternalInput").ap()
    xv_ap = nc.dram_tensor("xv_t", [QC, 128, ET, 512], BF16, kind="ExternalInput").ap()
    wq_ap = nc.dram_tensor("wq", [128, ET, DL], BF16, kind="ExternalInput").ap()
    wk_ap = nc.dram_tensor("wk", [128, ET, DL], BF16, kind="ExternalInput").ap()
    wv_ap = nc.dram_tensor("wv", [128, ET, HL * 65], BF16, kind="ExternalInput").ap()
    bq_ap = nc.dram_tensor("bq", [128, 2], F32, kind="ExternalInput").ap()
    bk_ap = nc.dram_tensor("bk", [128, 2], F32, kind="ExternalInput").ap()
    bv_ap = nc.dram_tensor("bv", [1, HL * 65], BF16, kind="ExternalInput").ap()
    wo_ap = nc.dram_tensor("wo", [128, 2, E], BF16, kind="ExternalInput").ap()
    out_ap = nc.dram_tensor("out_p", [S, E], F32, kind="ExternalOutput").ap()

    from contextlib import ExitStack
    with tile.TileContext(nc) as tc, ExitStack() as ctx:
        wpool = ctx.enter_context(tc.tile_pool(name="w", bufs=1))
        xtp = ctx.enter_context(tc.tile_pool(name="xt", bufs=5))
        big = ctx.enter_context(tc.tile_pool(name="big", bufs=1))
        expp = ctx.enter_context(tc.tile_pool(name="expp", bufs=4))
        znp = ctx.enter_context(tc.tile_pool(name="znp", bufs=2))
        smal = ctx.enter_context(tc.tile_pool(name="small", bufs=2))
        stg = ctx.enter_context(tc.tile_pool(name="stg", bufs=2))
        pscore = ctx.enter_context(tc.tile_pool(name="pscore", bufs=2, space="PSUM"))
        pav = ctx.enter_context(tc.tile_pool(name="pav", bufs=1, space="PSUM"))
        ptr = ctx.enter_context(tc.tile_pool(name="ptr", bufs=1, space="PSUM"))
        ppo = ctx.enter_context(tc.tile_pool(name="ppo", bufs=2, space="PSUM"))

        # ---- persistent weights / constants ----
        wq_sb = wpool.tile([128, ET, DL], BF16, tag="wq")
        wk_sb = wpool.tile([128, ET, DL], BF16, tag="wk")
        wv_sb = wpool.tile([128, ET, HL * 65], BF16, tag="wv")
        wo_sb = wpool.tile([128, 2, E], BF16, tag="wo")
        bq_sb = wpool.tile([128, 2], F32, tag="bq")
        bk_sb = wpool.tile([128, 2], F32, tag="bk")
        bv_sb = wpool.tile([1, HL * 65], BF16, tag="bv")
        ones_col = wpool.tile([1, 128], BF16, tag="ones")
        ident = wpool.tile([128, 128], BF16, tag="ident")

        nc.vector.memset(ones_col[:], 1.0)
        make_identity(nc, ident[:])

        QT = big.tile([128, NPAIR, S], BF16, tag="QT")
        KT = big.tile([128, NPAIR, S], BF16, tag="KT")
        Vones = big.tile([128, KT_TILES, HL, 65], BF16, tag="Vones")
        ZT = big.tile([128, NPAIR, S], BF16, tag="ZT")

        def load_xt_half(ap, sc, hf):
            # half of a 512-column slice of X^T: [128, 4 e-tiles, 512]
            t = xtp.tile([128, ET // 2, 512], BF16, tag="xt", name="xt")
            nc.sync.dma_start(t[:], ap[sc][:, hf * 4:(hf + 1) * 4, :])
            return t

        def load_xt_sc(ap, sc):
            return (load_xt_half(ap, sc, 0), load_xt_half(ap, sc, 1))

        def xe(x_pair, e):
            return x_pair[e // 4][:, e % 4, :]

        def proj_qk_sc(dst, w_sb, b_sb, x_sc, p, sc):
            # dst[:, p, sc-block] (transposed proj): out[d(128), s] = W.T @ X^T
            ps = ppo.tile([128, 512], F32, tag="ppo")
            for e in range(ET):
                nc.tensor.matmul(
                    ps[:], w_sb[:, e, p * 128:(p + 1) * 128], xe(x_sc, e),
                    start=(e == 0), stop=(e == ET - 1))
            nc.vector.tensor_scalar_add(
                dst[:, p, sc * 512:(sc + 1) * 512], ps[:], b_sb[:, p:p + 1])

        def gen_projqk(dst, w_sb, b_sb, x_ap, p):
            # generator: ~2 matmuls (0.45us) per pull
            for sc in range(QC):
                x_sc = load_xt_sc(x_ap, sc)
                ps = ppo.tile([128, 512], F32, tag="ppo", name="ps")
                for e in range(ET):
                    nc.tensor.matmul(
                        ps[:], w_sb[:, e, p * 128:(p + 1) * 128], xe(x_sc, e),
                        start=(e == 0), stop=(e == ET - 1))
                    if e % 2 == 1:
                        yield
                nc.vector.tensor_scalar_add(
                    dst[:, p, sc * 512:(sc + 1) * 512], ps[:], b_sb[:, p:p + 1])
                yield

        def gen_projv(x_ap):
            # single-pass V projection (all 4 heads, N=260), smeared
            for vsc in range(QC):
                x_sc = load_xt_sc(x_ap, vsc)
                for sti in range(4):
                    st = vsc * 4 + sti
                    ps = ppo.tile([128, HL * 65], F32, tag="ppo", name="ps")
                    for e in range(ET):
                        nc.tensor.matmul(
                            ps[:], xe(x_sc, e)[:, sti * 128:(sti + 1) * 128],
                            wv_sb[:, e, :],
                            start=(e == 0), stop=False)
                        if e % 2 == 1:
                            yield
                    nc.tensor.matmul(ps[:], ones_col[:], bv_sb[:],
                                     start=False, stop=True)
                    nc.vector.tensor_copy(
                        Vones[:, st], ps[:].rearrange("p (h d) -> p h d", h=HL))
                    yield

        def gen_av(p, qc, et):
            # AV + normalize + transpose for one (pair, q-chunk); ~0.3us per pull
            zn = znp.tile([128, 4, 2, D], BF16, tag="zn", name="zn")
            for h in range(2):
                avp = pav.tile([128, 4, 65], F32, tag="av", name="avp")
                # qt-outer: interleaved accumulation groups in one PSUM bank
                # are NOT allowed (each group's start clears the whole bank's
                # has_written bits) — a qt group must fully precede the next.
                for qt in range(4):
                    for kt in range(KT_TILES):
                        nc.tensor.matmul(
                            avp[:, qt, :],
                            et[:, kt, h, qt * 128:(qt + 1) * 128],
                            Vones[:, kt, 2 * p + h, :],
                            start=(kt == 0), stop=(kt == KT_TILES - 1))
                        if kt % 8 == 7:
                            yield
                rc = smal.tile([128, 4, 1], F32, tag="rc", name="rc")
                nc.vector.reciprocal(rc[:], avp[:, :, 64:65])
                nc.vector.tensor_mul(zn[:, :, h, :], avp[:, :, 0:D],
                                     rc[:].to_broadcast([128, 4, D]))
                yield
            for qt in range(4):
                tp = ptr.tile([128, 128], BF16, tag="tr", name="tp")
                nc.tensor.transpose(tp[:], zn[:, qt], ident[:])
                nc.vector.tensor_copy(
                    ZT[:, p, qc * 512 + qt * 128: qc * 512 + (qt + 1) * 128], tp[:])
                if qt % 2 == 1:
                    yield

        def gen_outproj(sts, act_evict=False):
            for st in sts:
                stt = stg.tile([128, 2, 512], F32, tag="stg", name="stt")
                for ec in range(2):
                    ps = ppo.tile([128, 512], F32, tag="ppo", name="ps")
                    for dt_ in range(2):
                        nc.tensor.matmul(
                            ps[:], ZT[:, dt_, st * 128:(st + 1) * 128],
                            wo_sb[:, dt_, ec * 512:(ec + 1) * 512],
                            start=(dt_ == 0), stop=(dt_ == 1))
                    if act_evict and ec == 1:
                        nc.scalar.copy(stt[:, ec], ps[:])
                    else:
                        nc.vector.tensor_copy(stt[:, ec], ps[:])
                    nc.sync.dma_start(
                        out_ap[st * 128:(st + 1) * 128, ec * 512:(ec + 1) * 512],
                        stt[:, ec])
                    yield

        def scores_kts(p, qc, et, kts):
            for kt in kts:
                sc_t = pscore.tile([128, 2, 512], F32, tag="sc")
                for h in range(2):
                    nc.tensor.matmul(
                        sc_t[:, h, :],
                        KT[64 * h:64 * (h + 1), p, kt * 128:(kt + 1) * 128],
                        QT[64 * h:64 * (h + 1), p, qc * 512:(qc + 1) * 512],
                        start=True, stop=True, tile_position=(64 * h, 0))
                nc.scalar.activation(et[:, kt], sc_t[:], AF.Exp)

        # ---- emission (static per-engine order ~ schedule priority) ----
        # PE warmup (HAM): dummy matmuls on a zeroed tile during the DMA lead-in
        warm = wpool.tile([128, 256], BF16, tag="warm")
        nc.vector.memset(warm[:], 0.0)
        wps = ppo.tile([128, 512], F32, tag="ppo")
        for i in range(16):
            nc.tensor.matmul(wps[:, 0:256], warm[:, 0:128], warm[:],
                             start=(i == 0), stop=(i == 15))

        # DMA order: xq-sc0, wq, xk-sc0, wk first so scores (-> exp) start ASAP
        nc.sync.dma_start(wq_sb[:], wq_ap[:])
        nc.sync.dma_start(wk_sb[:], wk_ap[:])

        # first q-chunk of scores interleaved with the QK projections
        et00 = expp.tile([128, KT_TILES, 2, 512], BF16, tag="expT")
        for sc in range(QC):
            xq_sc = load_xt_sc(xq_ap, sc)
            xk_sc = load_xt_sc(xk_ap, sc)
            if sc == 0:
                nc.sync.dma_start(bq_sb[:], bq_ap[:])
                nc.sync.dma_start(bk_sb[:], bk_ap[:])
            proj_qk_sc(QT, wq_sb, bq_sb, xq_sc, 0, sc)
            proj_qk_sc(KT, wk_sb, bk_sb, xk_sc, 0, sc)
            scores_kts(0, 0, et00, range(4 * sc, 4 * sc + 4))

        # V path loads (needed by first av phase)
        nc.sync.dma_start(wv_sb[:], wv_ap[:])
        nc.sync.dma_start(bv_sb[:], bv_ap[:])
        nc.sync.dma_start(wo_sb[:], wo_ap[:])

        def new_et():
            return expp.tile([128, KT_TILES, 2, 512], BF16, tag="expT", name="et")

        def drain(g, n=10 ** 9):
            """Pull generator g up to n times; True if exhausted."""
            for _ in range(n):
                if next(g, StopIteration) is StopIteration:
                    return True
            return False

        # background work generators, smeared between scores kt's (FIFO so only
        # one AV psum tile is live at a time)
        gv = gen_projv(xv_ap)
        gqt1 = gen_projqk(QT, wq_sb, bq_sb, xq_ap, 1)
        gkt1 = gen_projqk(KT, wk_sb, bk_sb, xk_ap, 1)
        ets = {(0, 0): et00}

        def run_loop(p, qc, fifo, budget):
            """Emit scores(p, qc) kt-by-kt, pulling `budget` steps per kt from
            the FIFO of filler generators."""
            et = ets.setdefault((p, qc), new_et())
            for kt in range(KT_TILES):
                scores_kts(p, qc, et, [kt])
                left = budget
                while left > 0 and fifo:
                    if drain(fifo[0], left):
                        fifo.pop(0)
                        left -= 1  # approximate
                    else:
                        left = 0

        # block-style emission: scores 4-kt blocks alternating with background
        # work blocks (~one vsc/sc group at a time)
        # loop 0: scores(0,1) + first half of V projection (2-kt granularity)
        et01 = ets.setdefault((0, 1), new_et())
        for i in range(8):
            scores_kts(0, 1, et01, range(2 * i, 2 * i + 2))
            drain(gv, 5)
        # loop 1: scores(0,2) + rest of V
        et02 = ets.setdefault((0, 2), new_et())
        for i in range(8):
            scores_kts(0, 2, et02, range(2 * i, 2 * i + 2))
            drain(gv, 5)
        drain(gv)
        # loop 2: scores(0,3) + QT1 + av(0,0)
        ga00 = gen_av(0, 0, ets[(0, 0)])
        et03 = ets.setdefault((0, 3), new_et())
        for i in range(8):
            scores_kts(0, 3, et03, range(2 * i, 2 * i + 2))
            drain(gqt1, 3)
            drain(ga00, 3)
        drain(gqt1)
        drain(ga00)
        # loop 3: scores(1,0) + KT1 (sc-block ordered ahead) + av(0,1)
        ga01 = gen_av(0, 1, ets[(0, 1)])
        et10 = ets.setdefault((1, 0), new_et())
        for sc in range(QC):
            drain(gkt1, 6)  # one full sc block of KT1
            scores_kts(1, 0, et10, range(4 * sc, 4 * sc + 2))
            drain(ga01, 3)
            scores_kts(1, 0, et10, range(4 * sc + 2, 4 * sc + 4))
            drain(ga01, 3)
        drain(gkt1)
        drain(ga01)
        # loop 4: scores(1,1) + av(1,0) + av(0,2)
        ga10 = gen_av(1, 0, ets[(1, 0)])
        ga02 = gen_av(0, 2, ets[(0, 2)])
        et11 = ets.setdefault((1, 1), new_et())
        for i in range(8):
            scores_kts(1, 1, et11, range(2 * i, 2 * i + 2))
            drain(ga10, 3)
            drain(ga02, 3)
        drain(ga10)
        drain(ga02)
        # loop 5: scores(1,2) + av(1,1) + av(0,3) + outproj(0-3)
        ga11 = gen_av(1, 1, ets[(1, 1)])
        ga03 = gen_av(0, 3, ets[(0, 3)])
        gop0 = gen_outproj(range(0, 4))
        et12 = ets.setdefault((1, 2), new_et())
        for i in range(8):
            scores_kts(1, 2, et12, range(2 * i, 2 * i + 2))
            drain(ga11, 3)
            drain(ga03, 3)
            drain(gop0, 1)
        drain(ga11)
        drain(ga03)
        # loop 6: scores(1,3) + av(1,2) + outproj(4-11)
        ga12 = gen_av(1, 2, ets[(1, 2)])
        gop1 = gen_outproj(range(4, 8))
        gop2 = gen_outproj(range(8, 12))
        et13 = ets.setdefault((1, 3), new_et())
        for i in range(8):
            scores_kts(1, 3, et13, range(2 * i, 2 * i + 2))
            drain(gop0, 1)
            drain(ga12, 3)
            drain(gop1, 1)
        drain(gop0)
        drain(ga12)
        drain(gop1)
        # tail: av(1,3) + outproj(8-15)
        ga13 = gen_av(1, 3, ets[(1, 3)])
        drain(ga13)
        gop3 = gen_outproj(range(12, 16), act_evict=True)
        drain(gop2)
        drain(gop3)

    nc.compile()
    return nc


def prep_inputs(query, key, value, Wq, bq, Wk, bk, Wv, bv, Wo, bo):
    """Host-side sharding: per-core input dicts (bf16, transposed/augmented)."""
    bf = ml_dtypes.bfloat16
    q32 = np.asarray(query, np.float32)
    k32 = np.asarray(key, np.float32)
    v32 = np.asarray(value, np.float32)
    Wq = np.asarray(Wq, np.float32)
    Wk = np.asarray(Wk, np.float32)
    Wv = np.asarray(Wv, np.float32)
    Wo = np.asarray(Wo, np.float32)
    bq = np.asarray(bq, np.float32)
    bk = np.asarray(bk, np.float32)
    bv = np.asarray(bv, np.float32)

    scale = 1.0 / np.sqrt(np.float32(D))

    def xt_layout(x2d):
        # [S, E] -> X^T [E, S] -> [sc, p, eo, j] contiguous tile layout
        a = x2d.T.reshape(ET, 128, QC, 512).transpose(2, 1, 0, 3)
        return np.ascontiguousarray(a).astype(bf)

    def w_layout(w2d):
        # [E, D'] -> [p, eo, D'] contiguous
        a = w2d.reshape(ET, 128, w2d.shape[1]).transpose(1, 0, 2)
        return np.ascontiguousarray(a).astype(bf)

    xt = {}
    for b in range(B):
        xt[('q', b)] = xt_layout(q32[b])
        xt[('k', b)] = xt_layout(k32[b])
        xt[('v', b)] = xt_layout(v32[b])

    in_maps = []
    for c in range(N_CORES):
        b, g = c // HL, c % HL
        hs = slice(g * DL, (g + 1) * DL)
        wv_aug = np.zeros((E, HL * 65), np.float32)
        bv_aug = np.zeros((1, HL * 65), np.float32)
        for h in range(HL):
            wv_aug[:, h * 65:h * 65 + D] = Wv[:, g * DL + h * D: g * DL + (h + 1) * D]
            bv_aug[0, h * 65:h * 65 + D] = bv[g * DL + h * D: g * DL + (h + 1) * D]
            bv_aug[0, h * 65 + D] = 1.0
        in_maps.append({
            "xq_t": xt[('q', b)],
            "xk_t": xt[('k', b)],
            "xv_t": xt[('v', b)],
            "wq": w_layout(Wq[:, hs] * scale),
            "wk": w_layout(Wk[:, hs]),
            "wv": w_layout(wv_aug),
            "bq": np.ascontiguousarray(
                (bq[hs] * scale).reshape(2, 128).T).astype(np.float32),
            "bk": np.ascontiguousarray(
                bk[hs].reshape(2, 128).T).astype(np.float32),
            "bv": bv_aug.astype(bf),
            "wo": np.ascontiguousarray(
                Wo[hs, :].reshape(2, 128, E).transpose(1, 0, 2)).astype(bf),
        })
    return in_maps


_NC_CACHE = [None]


def get_nc():
    if _NC_CACHE[0] is None:
        _install_ntff_hook()
        _NC_CACHE[0] = build_kernel()
    return _NC_CACHE[0]


def run(inputs, trace=False):
    nc = get_nc()
    in_maps = prep_inputs(**{k: v for k, v in inputs.items() if k != 'bo'},
                          bo=inputs['bo'])
    res = bass_utils.run_bass_kernel_spmd(
        nc, in_maps, core_ids=list(range(N_CORES)), trace=trace)
    bo = np.asarray(inputs['bo'], np.float32)
    out = np.empty((B, S, E), np.float32)
    for b in range(B):
        acc = np.zeros((S, E), np.float32)
        for g in range(HL):
            acc += res.results[b * HL + g]["out_p"]
        out[b] = acc + bo[None, :]
    return out, res


def kernel(**inputs):
    out, _ = run(inputs, trace=False)
    return out
